# revision 22
# baseline (speedup 1.0000x reference)
"""CollapseLoss kernel for Trainium2, 8-way row-sharded.

Reference computation (N=16384 rows, D=128):
    x_n   = row-normalize(feature_clusters)            # F.normalize(dim=1)
    d[i]  = dot(x_n[i+1], x_n[i])        i = 0..N-2
    out   = (d + 1/(N-1))**2

Sharding: 2048 rows per core. Host-side sharding materializes, per core, the
SBUF image xc[128, 17*128]: partition p holds rows 16p..16p+15 of the shard
(blocks 0..15) followed by row 16(p+1) (block 16 — the t=15 partner row,
which for p=127 is the next shard's first row: the halo).  Every device-side
load is then a plain contiguous column-range DMA, and the consecutive-row
dot for (16p+t, 16p+t+1) is a free-dim-shifted product:
    S[:, t] = sum_j AB[:, t*128+j] * AB[:, t*128+128+j]
(the t=15 partner sits right after block 15, so `in1` ranges stay
contiguous across the whole tile).

Work split (tuned against the TRN2 instruction cost model):
  DVE : fused product+reduce per block via scalar_tensor_tensor(accum_out),
        segmented reduces of Pool products / ACT squares, finals chain
  Pool: bulk shifted products for the early chunks (otherwise idle)
  ACT : Square+accum per block for mid blocks, Square passes for the rest

Tail avoids the inaccurate-rsqrt problem algebraically:
    (S/sqrt(m) + c)^2 == (S + c*sqrt(m))^2 / m      with m = na*nb
where sqrt(m) only scales the tiny c-term (c ~ 6.1e-5), so an integer-magic
sqrt approximation (one DVE tensor_scalar on the bitcast) is ample, and the
division uses the exact DVE iterative divide.  The ACT activation table is
loaded at t~0 via a dummy Square on a const AP so it never blocks the
stream.
"""

import numpy as np
from contextlib import ExitStack

N_ROWS = 16384
D = 128
N_CORES = 8
R = N_ROWS // N_CORES  # 2048 rows per core
P = 128                # partitions
Q = R // P             # 16 row-blocks per partition
C_CONST = 1.0 / (N_ROWS - 1)
SQRT_MAGIC = 0x1FBD1DF5  # bitcast(i>>1 + magic) ~= sqrt, rel err <= 4.5%

# tuning knobs (tuned against the TRN2 instruction cost model)
CFG = {
    # input DMA ranges in block units (block 16 = halo/partner), load order
    "load_order": ((0, 2), (2, 6), (6, 10), (10, 14), (14, 17)),
    # compute groups (products/norms emitted per group, in this order)
    "groups": ((0, 2), (2, 6), (6, 10), (10, 14), (14, 16)),
    "pool_groups": (),            # group indices: products via Pool TT
    "acc_blocks": (12, 13, 14, 15, 16),  # norms via ACT Square+accum
    "finals_groups": ((0, 16),),  # [start, end) output block ranges
    "act_sqrt": False,            # ACT table Sqrt vs DVE int-magic sqrt
    "dummy_square": True,         # hoist the ACT table load to t~0
}

_CACHE = {}


def _build_nc(cfg=None):
    import concourse.bacc as bacc
    import concourse.tile as tile
    from concourse import mybir

    cfg = dict(CFG, **(cfg or {}))
    f32 = mybir.dt.float32
    AF = mybir.ActivationFunctionType
    ALU = mybir.AluOpType
    acc_blocks = set(cfg["acc_blocks"])
    pool_groups = set(cfg["pool_groups"])
    fgroups = cfg["finals_groups"]

    nc = bacc.Bacc(
        "TRN2",
        target_bir_lowering=False,
        debug=False,
        enable_asserts=False,
        num_devices=N_CORES,
    )
    xc = nc.dram_tensor("xc", [P, (Q + 1) * D], f32, kind="ExternalInput").ap()
    out = nc.dram_tensor("out", [R], f32, kind="ExternalOutput").ap()
    out_pq = out.rearrange("(p q) -> p q", p=P)

    with tile.TileContext(nc) as tc:
        with ExitStack() as ctx:
            data = ctx.enter_context(tc.tile_pool(name="data", bufs=1))
            scr = ctx.enter_context(tc.tile_pool(name="scr", bufs=3))
            stat = ctx.enter_context(tc.tile_pool(name="stat", bufs=1))

            AB = data.tile([P, (Q + 1) * D], f32)
            PR = data.tile([P, Q * D], f32)   # pool products / ACT squares
            SQ = data.tile([P, Q * D], f32)
            S = stat.tile([P, Q], f32)        # raw consecutive-row dots
            NE = stat.tile([P, Q + 1], f32)   # squared norms incl. block 16

            if cfg["dummy_square"]:
                # hoist the single ACT table load to t~0; with act_sqrt the
                # dummy is a Sqrt so the set (sqrt_and_others) covers both
                dum = scr.tile([P, 1], f32, tag="dum")
                one = nc.const_aps.aps[(f32, 1.0)]
                dfn = AF.Sqrt if cfg["act_sqrt"] else AF.Square
                nc.scalar.activation(out=dum, in_=one[:P], func=dfn)

            for lo, hi in cfg["load_order"]:
                nc.sync.dma_start(out=AB[:, lo * D:hi * D],
                                  in_=xc[:, lo * D:hi * D])

            # halo norms first: the data lands first and NE[16] feeds finals
            sqb = scr.tile([P, D], f32, tag="sq")
            nc.scalar.activation(out=sqb, in_=AB[:, Q * D:(Q + 1) * D],
                                 func=AF.Square, accum_out=NE[:, Q:Q + 1])

            fired = set()
            for gidx, (ba, bb) in enumerate(cfg["groups"]):
                lo, hi = ba * D, bb * D
                # products (in1 spans one block past, contiguous incl. halo)
                if gidx in pool_groups:
                    nc.gpsimd.tensor_tensor(out=PR[:, lo:hi],
                                            in0=AB[:, lo:hi],
                                            in1=AB[:, lo + D:hi + D],
                                            op=ALU.mult)
                    nc.vector.tensor_reduce(
                        S[:, ba:bb],
                        PR[:, lo:hi].rearrange("p (q d) -> p q d", q=bb - ba),
                        axis=mybir.AxisListType.X, op=ALU.add)
                else:
                    for t in range(ba, bb):
                        blk = AB[:, t * D:(t + 1) * D]
                        nxt = AB[:, (t + 1) * D:(t + 2) * D]
                        pr = scr.tile([P, D], f32, tag="pr", name=f"pr{t}")
                        nc.vector.scalar_tensor_tensor(
                            out=pr, in0=blk, scalar=1.0, in1=nxt,
                            op0=ALU.bypass, op1=ALU.mult,
                            accum_out=S[:, t:t + 1])

                # norms
                for t in [t for t in range(ba, bb) if t in acc_blocks]:
                    blk = AB[:, t * D:(t + 1) * D]
                    sqt = scr.tile([P, D], f32, tag="sq", name=f"sqa{t}")
                    nc.scalar.activation(out=sqt, in_=blk, func=AF.Square,
                                         accum_out=NE[:, t:t + 1])
                run = []
                for t in [t for t in range(ba, bb)
                          if t not in acc_blocks] + [None]:
                    if run and (t is None or t != run[-1] + 1):
                        a, b = run[0], run[-1] + 1
                        nc.scalar.activation(out=SQ[:, a * D:b * D],
                                             in_=AB[:, a * D:b * D],
                                             func=AF.Square)
                        nc.vector.tensor_reduce(
                            NE[:, a:b],
                            SQ[:, a * D:b * D].rearrange(
                                "p (q d) -> p q d", q=b - a),
                            axis=mybir.AxisListType.X, op=ALU.add)
                        run = []
                    if t is not None:
                        run.append(t)

                # finals for any output group now fully determined
                done = bb
                for gi, (ga, gb) in enumerate(fgroups):
                    need = gb + 1 if gb < Q else Q
                    if gi not in fired and need <= done:
                        fired.add(gi)
                        _emit_finals(nc, stat, mybir, S, NE, out_pq,
                                     ga, gb, gi, cfg)

    nc.compile()
    return nc


def _emit_finals(nc, stat, mybir, S, NE, out_pq, ga, gb, gi, cfg):
    """out[:, ga:gb] = (S + c*sqrt(m))^2 / m for block range [ga, gb)."""
    ALU = mybir.AluOpType
    f32 = mybir.dt.float32
    i32 = mybir.dt.int32
    AF = mybir.ActivationFunctionType
    w_ = gb - ga
    m = stat.tile([P, w_], f32, name=f"m{gi}")
    nc.vector.tensor_tensor(out=m, in0=NE[:, ga:gb], in1=NE[:, ga + 1:gb + 1],
                            op=ALU.mult)
    w = stat.tile([P, w_], f32, name=f"w{gi}")
    nc.vector.reciprocal(w, m)   # off the sqrt chain; joins at the end
    s0 = stat.tile([P, w_], f32, name=f"s0{gi}")
    if cfg["act_sqrt"]:
        nc.scalar.activation(out=s0, in_=m, func=AF.Sqrt)
    else:
        sh = stat.tile([P, w_], f32, name=f"sh{gi}")
        nc.vector.tensor_scalar(sh.bitcast(i32), m.bitcast(i32), 1, None,
                                ALU.logical_shift_right)
        nc.vector.tensor_scalar(s0.bitcast(i32), sh.bitcast(i32), SQRT_MAGIC,
                                None, ALU.add)
    u = stat.tile([P, w_], f32, name=f"u{gi}")
    nc.vector.scalar_tensor_tensor(out=u, in0=s0, scalar=C_CONST,
                                   in1=S[:, ga:gb], op0=ALU.mult, op1=ALU.add)
    v = stat.tile([P, w_], f32, name=f"v{gi}")
    nc.vector.tensor_tensor(out=v, in0=u, in1=u, op=ALU.mult)
    o = stat.tile([P, w_], f32, name=f"o{gi}")
    nc.vector.tensor_tensor(out=o, in0=v, in1=w, op=ALU.mult)
    nc.sync.dma_start(out=out_pq[:, ga:gb], in_=o)


def _get_nc():
    if "nc" not in _CACHE:
        _CACHE["nc"] = _build_nc()
    return _CACHE["nc"]


def make_in_maps(x: np.ndarray) -> list[dict[str, np.ndarray]]:
    """Host-side sharding: build each core's SBUF image xc[128, 2176]."""
    x = np.ascontiguousarray(np.asarray(x, dtype=np.float32))
    # pad one row (the out-of-range halo of the last core) with ones
    xp = np.concatenate([x, np.ones((1, D), dtype=np.float32)], axis=0)
    in_maps = []
    for c in range(N_CORES):
        sh = xp[c * R:c * R + R].reshape(P, Q * D)        # blocks 0..15
        halo = xp[c * R + 16 * np.arange(1, P + 1)]       # block 16
        xc = np.concatenate([sh, halo.reshape(P, D)], axis=1)
        in_maps.append({"xc": np.ascontiguousarray(xc)})
    return in_maps


def kernel(feature_clusters: np.ndarray) -> np.ndarray:
    from concourse.bass_utils import run_bass_kernel_spmd

    nc = _get_nc()
    in_maps = make_in_maps(feature_clusters)
    res = run_bass_kernel_spmd(nc, in_maps, list(range(N_CORES))).results
    full = np.concatenate([res[c]["out"] for c in range(N_CORES)])
    return full[:N_ROWS - 1].astype(np.float32)
